# revision 9
# baseline (speedup 1.0000x reference)
"""DecoderLSTM Trainium2 kernel.

Computes, for inputs matching the reference nn module:
    x  = embed_table[captions]                      # [B, T, E]
    xg = einsum('bte,ge->tbg', x, W_ih) + b_ih + b_hh
    (h, c) LSTM scan over T steps, h0 = features, c0 = 0
    out = einsum('tbh,vh->btv', hs, W_out) + b_out  # [B, T, V]

Sharding: data-parallel over batch. 8 cores x 16 batch rows each.
Weights are replicated (cast to bf16 host-side); each core computes its
16-row slice of the output. Per-core output is produced in transposed
layout [V, T*Bc] and untransposed on the host during unshard.

Device layout notes (per core, Bc = 16 batch rows):
  - Embedding gather: dma_gather(transpose=True) pulls the 320 caption
    rows of the bf16 [V, 384]-padded table directly into x_T layout
    [128p=E-offset, 3=E-block, 384=(t,b) col]. Table column 383 is 1.0
    so row 383 of W_ihT carries (b_ih + b_hh): bias folded into the
    xg matmul.
  - Gate permutation: the 4H=2048 gate dim is reordered host-side so
    column-group j of the recurrent matmul computes
    [i_j | f_j | g_j | o_j] (H-slice j of each gate). Gates land in one
    PSUM bank as [128=(32j+b), 4, 128] and the whole nonlinearity runs
    on [128, *] tiles.
  - Recurrent matmul: 4 concurrent column-tiled matmuls (tile_position
    (0, 32j), M=16) stream W_hhT chunks; each group's accumulation is
    seeded by an identity-matmul that injects xg_t (start=True).
  - h_T for the next step comes from 4 row-tiled PE transposes of the
    [16@32k, 128] slices of h.
  - Projection: out_T = W_out @ hs_T with W_out blocks stationary,
    V on partitions; b_out added during PSUM evacuation via the ACT
    per-partition bias. Chunked in two so the first half overlaps the
    recurrence.
"""

import numpy as np
import ml_dtypes

import concourse.bass as bass
import concourse.mybir as mybir
import concourse.tile as tile
from concourse import bacc

BF16 = mybir.dt.bfloat16
F32 = mybir.dt.float32
I16 = mybir.dt.int16

B, T, E, H, V = 128, 20, 300, 512, 10000
EPAD = 384            # E padded; col 383 is the ones column (bias row)
NCORES = 8
BC = B // NCORES      # 16 batch rows per core
NT = BC * T           # 320 (t,b) columns per core
NIDX = 384            # gather idx count (padded to %128)
NV = 79               # ceil(10112 / 128) vocab row-tiles
VPAD = NV * 128       # 10112
AF = mybir.ActivationFunctionType


def _gate_perm():
    """new gate-dim order: chunk j = [i_j | f_j | g_j | o_j], blocks of 128."""
    perm = np.empty(4 * H, dtype=np.int64)
    n = 0
    for j in range(4):
        for q in range(4):          # i, f, g, o (PyTorch LSTM order)
            for r in range(128):
                perm[n] = q * H + j * 128 + r
                n += 1
    return perm


def build_nc():
    nc = bacc.Bacc("TRN2", target_bir_lowering=False, debug=False)

    # ---- DRAM parameters (per-core shapes) ----
    emb_d = nc.dram_tensor("emb", [V, EPAD], BF16, kind="ExternalInput")
    idx_d = nc.dram_tensor("idx", [128, NIDX // 16], I16, kind="ExternalInput")
    wih_d = nc.dram_tensor("wih", [3, 128, 4, 512], BF16, kind="ExternalInput")
    whh_d = nc.dram_tensor("whh", [4, 128, 4, 512], BF16, kind="ExternalInput")
    wout_d = nc.dram_tensor("wout", [4, 128, NV, 128], BF16, kind="ExternalInput")
    bout_d = nc.dram_tensor("bout", [128, NV], F32, kind="ExternalInput")
    h0t_d = nc.dram_tensor("h0t", [128, 4, BC], BF16, kind="ExternalInput")
    idf_d = nc.dram_tensor("idf", [128, 128], F32, kind="ExternalInput")
    i16b_d = nc.dram_tensor("i16b", [16, 16], BF16, kind="ExternalInput")
    outT_d = nc.dram_tensor("outT", [NV, 128, NT], F32, kind="ExternalOutput")

    with tile.TileContext(nc) as tc:
        with (
            tc.tile_pool(name="const", bufs=1) as const,
            tc.tile_pool(name="wpool", bufs=1) as wpool,
            tc.tile_pool(name="xgstep", bufs=3) as xgstep_p,
            tc.tile_pool(name="work", bufs=2) as work,
            tc.tile_pool(name="stage", bufs=4) as stage_p,
            tc.tile_pool(name="psg", bufs=1, space="PSUM") as ps_gates,
            tc.tile_pool(name="psh", bufs=1, space="PSUM") as ps_ht,
            tc.tile_pool(name="psb", bufs=2, space="PSUM") as ps_big,
        ):
            idx_sb = const.tile([128, NIDX // 16], I16, tag="idx")
            i16b_sb = const.tile([16, 16], BF16, tag="i16b")
            idf_sb = const.tile([128, 128], F32, tag="idf")
            bout_sb = const.tile([128, NV], F32, tag="bout")

            xT = wpool.tile([128, 3, EPAD], BF16, tag="xT")
            wih_sb = [wpool.tile([128, 4, 512], BF16, tag=f"wih{k}", name=f"wih{k}") for k in range(3)]
            whh_sb = [wpool.tile([128, 4, 512], BF16, tag=f"whh{k}", name=f"whh{k}") for k in range(4)]
            wout_sb = [wpool.tile([128, NV, 128], BF16, tag=f"wo{k}", name=f"wo{k}") for k in range(4)]
            xg_sb = [wpool.tile([128, 4, 512], BF16, tag=f"xg{m}", name=f"xg{m}") for m in range(3)]
            # hs_T: slot s holds h after step s-1 (slot 0 = h0), packed
            # [128p=H-offset, slot, H-block k, b]
            hsT = wpool.tile([128, T + 1, 4, BC], BF16, tag="hsT")
            C = wpool.tile([128, 128], F32, tag="C")

            gates_ps = ps_gates.tile([128, 4, 128], F32, tag="g")
            ht_ps = ps_ht.tile([128, 4, 32], F32, tag="ht")

            # ---- loads ----
            nc.sync.dma_start(idx_sb[:], idx_d[:])
            nc.sync.dma_start(i16b_sb[:], i16b_d[:])
            nc.sync.dma_start(idf_sb[:], idf_d[:])
            nc.sync.dma_start(bout_sb[:], bout_d[:])
            for k in range(3):
                nc.sync.dma_start(wih_sb[k][:], wih_d[k])
            for k in range(4):
                nc.sync.dma_start(whh_sb[k][:], whh_d[k])
            nc.sync.dma_start(hsT[:, 0, :, :], h0t_d[:])
            nc.vector.memset(gates_ps[:], 0.0)
            nc.vector.memset(C[:], 0.0)
            nc.gpsimd.dma_gather(xT[:], emb_d[:], idx_sb[:], NIDX, NIDX, EPAD,
                                 transpose=True)
            for k in range(4):
                nc.sync.dma_start(wout_sb[k][:], wout_d[k])

            # ---- xg = x @ W_ihT  -> [(t,b) rows, 2048 perm'd gate cols] ----
            for m in range(3):
                for n in range(4):
                    ps = ps_big.tile([128, 512], F32, tag="big")
                    for k in range(3):
                        nc.tensor.matmul(
                            ps[:],
                            xT[:, k, m * 128:(m + 1) * 128],
                            wih_sb[k][:, n, :],
                            start=(k == 0), stop=(k == 2),
                        )
                    if n % 2 == 0:
                        nc.vector.tensor_copy(xg_sb[m][:, n, :], ps[:])
                    else:
                        nc.scalar.copy(xg_sb[m][:, n, :], ps[:])

            # ---- projection emitter (V rows on partitions) ----
            def emit_proj(v, s0, s1):
                w = (s1 - s0) * BC
                pp = ps_big.tile([128, w], F32, tag="big")
                for k in range(4):
                    nc.tensor.matmul(
                        pp[:],
                        wout_sb[k][:, v, :],
                        hsT[:, s0:s1, k, :],
                        start=(k == 0), stop=(k == 3),
                    )
                st = stage_p.tile([128, w], F32, tag="st")
                if v % 2 == 0:
                    nc.scalar.activation(st[:], pp[:], AF.Identity,
                                         bias=bout_sb[:, v:v + 1])
                else:
                    nc.vector.tensor_scalar_add(st[:], pp[:], bout_sb[:, v:v + 1])
                nc.sync.dma_start(outT_d[v, :, (s0 - 1) * BC:(s1 - 1) * BC], st[:])

            # ---- recurrence ----
            for t in range(T):
                m, r0 = t // 8, (t % 8) * BC
                # rebase this step's xg rows to partition base 0 (prefetchable)
                xg_t = xgstep_p.tile([BC, 4, 512], BF16, tag="xgt")
                nc.sync.dma_start(xg_t[:], xg_sb[m][r0:r0 + BC, :, :])

                # gate matmuls: 4 concurrent column groups
                for j in range(4):
                    nc.tensor.matmul(
                        gates_ps[32 * j:32 * j + BC, :, :],
                        i16b_sb[:],
                        xg_t[:, j, :],
                        start=True, stop=False,
                        tile_position=(0, 32 * j),
                        skip_group_check=True,
                    )
                for k in range(4):
                    for j in range(4):
                        nc.tensor.matmul(
                            gates_ps[32 * j:32 * j + BC, :, :],
                            hsT[:, t, k, :],
                            whh_sb[k][:, j, :],
                            start=False, stop=(k == 3),
                            tile_position=(0, 32 * j),
                            skip_group_check=True,
                        )

                # nonlinearity: A = [sig(i), sig(f), tanh(g), sig(o)]
                A = work.tile([128, 4, 128], F32, tag="A")
                nc.scalar.activation(A[:, 0:2, :], gates_ps[:, 0:2, :], AF.Sigmoid)
                nc.scalar.activation(A[:, 2, :], gates_ps[:, 2, :], AF.Tanh)
                nc.scalar.activation(A[:, 3, :], gates_ps[:, 3, :], AF.Sigmoid)
                T2 = work.tile([128, 128], F32, tag="T2")
                T1 = work.tile([128, 128], F32, tag="T1")
                TC = work.tile([128, 128], F32, tag="TC")
                Hn = work.tile([128, 128], F32, tag="Hn")
                nc.vector.tensor_mul(T2[:], A[:, 0, :], A[:, 2, :])   # i*g
                nc.vector.tensor_mul(T1[:], A[:, 1, :], C[:])         # f*c
                nc.vector.tensor_add(C[:], T1[:], T2[:])
                nc.scalar.activation(TC[:], C[:], AF.Tanh)
                nc.vector.tensor_mul(Hn[:], A[:, 3, :], TC[:])        # o*tanh(c)

                # h -> h_T: one full 128x128 PE transpose; the four h_T
                # blocks are ht_ps[:, k, 0:16] (garbage lands in cols 16:32)
                nc.tensor.transpose(ht_ps[:], Hn[:], idf_sb[:])
                nc.vector.tensor_copy(hsT[:, t + 1, :, :], ht_ps[:, :, 0:BC])

                # overlap first projection chunk (slots 1..10) with steps 10+
                if t >= 10:
                    v0 = (t - 10) * 9
                    for v in range(v0, min(v0 + 9, NV)):
                        emit_proj(v, 1, 11)

            for v in range(81, NV):   # leftovers of chunk 0 (none for NV=79)
                emit_proj(v, 1, 11)
            for v in range(NV):       # chunk 1: slots 11..20
                emit_proj(v, 11, 21)

    nc.compile()
    return nc


def prep_inputs(features, captions, embed_table, W_ih, W_hh, b_ih, b_hh,
                W_out, b_out):
    """Host-side shard + layout prep. Returns per-core input maps."""
    bf = ml_dtypes.bfloat16
    features = np.asarray(features, dtype=np.float32)
    captions = np.asarray(captions).astype(np.int64)
    embed_table = np.asarray(embed_table, dtype=np.float32)
    W_ih = np.asarray(W_ih, dtype=np.float32)
    W_hh = np.asarray(W_hh, dtype=np.float32)
    b_ih = np.asarray(b_ih, dtype=np.float32)
    b_hh = np.asarray(b_hh, dtype=np.float32)
    W_out = np.asarray(W_out, dtype=np.float32)
    b_out = np.asarray(b_out, dtype=np.float32)

    perm = _gate_perm()

    emb = np.zeros((V, EPAD), dtype=bf)
    emb[:, :E] = embed_table.astype(bf)
    emb[:, EPAD - 1] = bf(1.0)

    wih = np.zeros((EPAD, 4 * H), dtype=np.float32)
    wih[:E, :] = W_ih.T[:, perm]
    wih[EPAD - 1, :] = (b_ih + b_hh)[perm]
    wih = wih.astype(bf).reshape(3, 128, 4, 512)

    whh = np.ascontiguousarray(W_hh.T[:, perm]).astype(bf).reshape(4, 128, 4, 512)

    wout = np.zeros((H, VPAD), dtype=np.float32)
    wout[:, :V] = W_out.T
    wout = wout.astype(bf).reshape(4, 128, NV, 128)

    boutp = np.zeros((VPAD,), dtype=np.float32)
    boutp[:V] = b_out
    bout_r = np.ascontiguousarray(boutp.reshape(NV, 128).T)

    idf = np.eye(128, dtype=np.float32)
    i16b = np.eye(16, dtype=bf)

    shared = dict(emb=emb, wih=wih, whh=whh, wout=wout, bout=bout_r,
                  idf=idf, i16b=i16b)

    in_maps = []
    for c in range(NCORES):
        cap_c = captions[c * BC:(c + 1) * BC]                 # [16, 20]
        # idx block [16, NIDX//16], replicated into all 8 GpSimd core groups
        blk = np.zeros((16, NIDX // 16), dtype=np.int16)
        blk[:, :T] = cap_c.astype(np.int16)
        idx = np.tile(blk, (8, 1))
        feat_c = features[c * BC:(c + 1) * BC]                # [16, 512]
        h0t = np.ascontiguousarray(
            feat_c.reshape(BC, 4, 128).transpose(2, 1, 0)).astype(bf)
        in_maps.append(dict(shared, idx=idx, h0t=h0t))
    return in_maps


def unshard(core_outs):
    """core_outs: list of 8 arrays [NV, 128, NT] f32 -> full [B, T, V]."""
    parts = []
    for o in core_outs:
        o = np.asarray(o, dtype=np.float32).reshape(VPAD, NT)[:V]  # [V, 320]
        parts.append(o.reshape(V, T, BC).transpose(2, 1, 0))       # [16, T, V]
    return np.ascontiguousarray(np.concatenate(parts, axis=0))


_NC_CACHE = {}


def kernel(**inputs) -> np.ndarray:
    from concourse.bass_utils import run_bass_kernel_spmd

    if "nc" not in _NC_CACHE:
        _NC_CACHE["nc"] = build_nc()
    nc = _NC_CACHE["nc"]

    in_maps = prep_inputs(**inputs)
    res = run_bass_kernel_spmd(nc, in_maps, core_ids=list(range(NCORES)))
    return unshard([res.results[c]["outT"] for c in range(NCORES)])


# revision 14
# speedup vs baseline: 1.0574x; 1.0574x over previous
"""DecoderLSTM Trainium2 kernel.

Computes, for inputs matching the reference nn module:
    x  = embed_table[captions]                      # [B, T, E]
    xg = einsum('bte,ge->tbg', x, W_ih) + b_ih + b_hh
    (h, c) LSTM scan over T steps, h0 = features, c0 = 0
    out = einsum('tbh,vh->btv', hs, W_out) + b_out  # [B, T, V]

Sharding: data-parallel over batch. 8 cores x 16 batch rows each.
Weights are replicated (cast to bf16 host-side); each core computes its
16-row slice of the output. Per-core output is produced in transposed
layout [V, T*Bc] and untransposed on the host during unshard.

Device layout notes (per core, Bc = 16 batch rows):
  - Embedding gather: dma_gather(transpose=True) pulls the 320 caption
    rows of the bf16 [V, 384]-padded table directly into x_T layout
    [128p=E-offset, 3=E-block, 384=(t,b) col]. Table column 383 is 1.0
    so row 383 of W_ihT carries (b_ih + b_hh): bias folded into the
    xg matmul.
  - Gate permutation: the 4H=2048 gate dim is reordered host-side so
    column-group j of the recurrent matmul computes
    [i_j | f_j | g_j | o_j] (H-slice j of each gate). Gates land in one
    PSUM bank as [128=(32j+b), 4, 128] and the whole nonlinearity runs
    on [128, *] tiles.
  - Recurrent matmul: 4 concurrent column-tiled matmuls (tile_position
    (0, 32j), M=16) stream W_hhT chunks; each group's accumulation is
    seeded by an identity-matmul that injects xg_t (start=True).
  - h_T for the next step comes from 4 row-tiled PE transposes of the
    [16@32k, 128] slices of h.
  - Projection: out_T = W_out @ hs_T with W_out blocks stationary,
    V on partitions; b_out added during PSUM evacuation via the ACT
    per-partition bias. Chunked in two so the first half overlaps the
    recurrence.
"""

import numpy as np
import ml_dtypes

import concourse.bass as bass
import concourse.mybir as mybir
import concourse.tile as tile
from concourse import bacc

BF16 = mybir.dt.bfloat16
F32 = mybir.dt.float32
I16 = mybir.dt.int16

B, T, E, H, V = 128, 20, 300, 512, 10000
EPAD = 384            # E padded; col 383 is the ones column (bias row)
NCORES = 8
BC = B // NCORES      # 16 batch rows per core
NT = BC * T           # 320 (t,b) columns per core
NIDX = 384            # gather idx count (padded to %128)
NV = 79               # ceil(10112 / 128) vocab row-tiles
VPAD = NV * 128       # 10112
AF = mybir.ActivationFunctionType


def _gate_perm():
    """new gate-dim order: chunk j = [i_j | f_j | g_j | o_j], blocks of 128."""
    perm = np.empty(4 * H, dtype=np.int64)
    n = 0
    for j in range(4):
        for q in range(4):          # i, f, g, o (PyTorch LSTM order)
            for r in range(128):
                perm[n] = q * H + j * 128 + r
                n += 1
    return perm


def build_nc():
    nc = bacc.Bacc("TRN2", target_bir_lowering=False, debug=False)

    # ---- DRAM parameters (per-core shapes) ----
    emb_d = nc.dram_tensor("emb", [V, EPAD], BF16, kind="ExternalInput")
    idx_d = nc.dram_tensor("idx", [128, NIDX // 16], I16, kind="ExternalInput")
    wih_d = nc.dram_tensor("wih", [3, 128, 4, 512], BF16, kind="ExternalInput")
    whh_d = nc.dram_tensor("whh", [4, 128, 4, 512], BF16, kind="ExternalInput")
    wout_d = nc.dram_tensor("wout", [4, 128, NV, 128], BF16, kind="ExternalInput")
    bout_d = nc.dram_tensor("bout", [128, NV], F32, kind="ExternalInput")
    h0t_d = nc.dram_tensor("h0t", [128, 4, BC], BF16, kind="ExternalInput")
    idf_d = nc.dram_tensor("idf", [128, 128], F32, kind="ExternalInput")
    i16b_d = nc.dram_tensor("i16b", [16, 16], BF16, kind="ExternalInput")
    outT_d = nc.dram_tensor("outT", [128, NV, NT], F32, kind="ExternalOutput")

    with tile.TileContext(nc) as tc:
        with (
            tc.tile_pool(name="const", bufs=1) as const,
            tc.tile_pool(name="wpool", bufs=1) as wpool,
            tc.tile_pool(name="xgstep", bufs=3) as xgstep_p,
            tc.tile_pool(name="work", bufs=2) as work,
            tc.tile_pool(name="stage", bufs=4) as stage_p,
            tc.tile_pool(name="psg", bufs=1, space="PSUM") as ps_gates,
            tc.tile_pool(name="psh", bufs=1, space="PSUM") as ps_ht,
            tc.tile_pool(name="psb", bufs=2, space="PSUM") as ps_big,
            tc.tile_pool(name="psj", bufs=1, space="PSUM") as ps_junk,
        ):
            idx_sb = const.tile([128, NIDX // 16], I16, tag="idx")
            i16b_sb = const.tile([16, 16], BF16, tag="i16b")
            idf_sb = const.tile([128, 128], F32, tag="idf")
            bout_sb = const.tile([128, NV], F32, tag="bout")

            xT = wpool.tile([128, 3, EPAD], BF16, tag="xT")
            wih_sb = [wpool.tile([128, 4, 512], BF16, tag=f"wih{k}", name=f"wih{k}") for k in range(3)]
            whh_sb = [wpool.tile([128, 4, 512], BF16, tag=f"whh{k}", name=f"whh{k}") for k in range(4)]
            wout_sb = [wpool.tile([128, NV, 128], BF16, tag=f"wo{k}", name=f"wo{k}") for k in range(4)]
            xg_sb = [wpool.tile([128, 4, 512], BF16, tag=f"xg{m}", name=f"xg{m}") for m in range(3)]
            # hs_T: slot s holds h after step s-1 (slot 0 = h0), packed
            # [128p=H-offset, slot, H-block k, b]
            hsT = wpool.tile([128, T + 1, 4, BC], BF16, tag="hsT")
            C = wpool.tile([128, 128], F32, tag="C")

            gates_ps = ps_gates.tile([128, 4, 128], F32, tag="g")
            ht_ps = ps_ht.tile([128, 4, 32], F32, tag="ht")
            junk_ps = ps_junk.tile([16, 512], F32, tag="junk")

            # ---- loads ----
            # sync HWDGE ring: only small latency-critical transfers (idx,
            # h0t, per-step xg rebases). scalar HWDGE ring: bulk weights.
            # HWDGE executes FIFO per issuing engine, so big weight loads
            # must not sit in front of the per-step rebase DMAs.
            nc.sync.dma_start(idx_sb[:], idx_d[:])
            nc.sync.dma_start(hsT[:, 0, :, :], h0t_d[:])
            nc.gpsimd.dma_gather(xT[:], emb_d[:], idx_sb[:], NIDX, NIDX, EPAD,
                                 transpose=True)
            nc.scalar.dma_start(i16b_sb[:], i16b_d[:])
            nc.scalar.dma_start(idf_sb[:], idf_d[:])
            nc.scalar.dma_start(bout_sb[:], bout_d[:])
            for k in range(3):
                nc.scalar.dma_start(wih_sb[k][:], wih_d[k])
            for k in range(4):
                nc.scalar.dma_start(whh_sb[k][:], whh_d[k])
            nc.vector.memset(gates_ps[:], 0.0)
            nc.vector.memset(C[:], 0.0)
            for k in range(4):
                nc.scalar.dma_start(wout_sb[k][:], wout_d[k])

            # ---- xg = x @ W_ihT -> [(t,b) rows, 2048 perm'd gate cols] ----
            # m-tile 0 runs up front (needed at step 0); m1/m2 n-groups are
            # emitted inside steps 0..7 as PE filler during the act windows.
            def emit_xg(m, n):
                ps = ps_big.tile([128, 512], F32, tag="big")
                for k in range(3):
                    nc.tensor.matmul(
                        ps[:],
                        xT[:, k, m * 128:(m + 1) * 128],
                        wih_sb[k][:, n, :],
                        start=(k == 0), stop=(k == 2),
                    )
                if n % 2 == 0:
                    nc.vector.tensor_copy(xg_sb[m][:, n, :], ps[:])
                else:
                    nc.scalar.copy(xg_sb[m][:, n, :], ps[:])

            for n in range(4):
                emit_xg(0, n)

            def emit_xgt_fetch(t):
                # rebase step-t xg rows to partition base 0 (prefetched)
                m, r0 = t // 8, (t % 8) * BC
                xg_t = xgstep_p.tile([BC, 4, 512], BF16, tag="xgt",
                                     name=f"xgt{t}")
                nc.sync.dma_start(xg_t[:], xg_sb[m][r0:r0 + BC, :, :])
                return xg_t

            def emit_junk(n_mms):
                # dependency-free matmuls that keep the PE HAM busy during
                # the activation window so the clock gate stays at 2.4 GHz
                for _ in range(n_mms):
                    nc.tensor.matmul(junk_ps[:], hsT[:, 0, 0, :],
                                     whh_sb[0][:, 0, :],
                                     start=True, stop=True,
                                     skip_group_check=True)

            # ---- recurrence ----
            xg_fetched = [emit_xgt_fetch(0), emit_xgt_fetch(1)]
            for t in range(T):
                xg_t = xg_fetched[t]
                if t + 2 < T:
                    xg_fetched.append(emit_xgt_fetch(t + 2))

                # gate matmuls: 4 concurrent column groups
                for j in range(4):
                    nc.tensor.matmul(
                        gates_ps[32 * j:32 * j + BC, :, :],
                        i16b_sb[:],
                        xg_t[:, j, :],
                        start=True, stop=False,
                        tile_position=(0, 32 * j),
                        skip_group_check=True,
                    )
                for k in range(4):
                    for j in range(4):
                        nc.tensor.matmul(
                            gates_ps[32 * j:32 * j + BC, :, :],
                            hsT[:, t, k, :],
                            whh_sb[k][:, j, :],
                            start=False, stop=(k == 3),
                            tile_position=(0, 32 * j),
                            skip_group_check=True,
                        )

                # nonlinearity: A = [sig(i), sig(f), tanh(g), sig(o)]
                A = work.tile([128, 4, 128], F32, tag="A")
                nc.scalar.activation(A[:, 0:2, :], gates_ps[:, 0:2, :], AF.Sigmoid)
                nc.scalar.activation(A[:, 2, :], gates_ps[:, 2, :], AF.Tanh)
                nc.scalar.activation(A[:, 3, :], gates_ps[:, 3, :], AF.Sigmoid)
                T2 = work.tile([128, 128], F32, tag="T2")
                T1 = work.tile([128, 128], F32, tag="T1")
                TC = work.tile([128, 128], F32, tag="TC")
                Hn = work.tile([128, 128], F32, tag="Hn")
                nc.vector.tensor_mul(T2[:], A[:, 0, :], A[:, 2, :])   # i*g
                nc.vector.tensor_mul(T1[:], A[:, 1, :], C[:])         # f*c
                nc.vector.tensor_add(C[:], T1[:], T2[:])
                nc.scalar.activation(TC[:], C[:], AF.Tanh)
                nc.vector.tensor_mul(Hn[:], A[:, 3, :], TC[:])        # o*tanh(c)

                # PE filler during the act window: xg m1/m2 early, junk after
                if t < 4:
                    emit_xg(1, t)
                    emit_junk(2)
                elif t < 8:
                    emit_xg(2, t - 4)
                    emit_junk(2)
                elif t < T - 1:
                    emit_junk(8)

                # h -> h_T: one full 128x128 PE transpose; the four h_T
                # blocks are ht_ps[:, k, 0:16] (garbage lands in cols 16:32)
                nc.tensor.transpose(ht_ps[:], Hn[:], idf_sb[:])
                nc.vector.tensor_copy(hsT[:, t + 1, :, :], ht_ps[:, :, 0:BC])

            # ---- projection tail: out_T = W_out @ hs_T, all 20 slots ----
            # (N=320 streams keep LDWEIGHTS fully hidden; PE is warm here)
            st = None
            for v in range(NV):
                pp = ps_big.tile([128, NT], F32, tag="big")
                for k in range(4):
                    nc.tensor.matmul(
                        pp[:],
                        wout_sb[k][:, v, :],
                        hsT[:, 1:T + 1, k, :],
                        start=(k == 0), stop=(k == 3),
                    )
                g = v % 4
                if g == 0:
                    nv = min(4, NV - v)
                    st = stage_p.tile([128, nv, NT], F32, tag="st",
                                      name=f"st{v}")
                if v % 2 == 0:
                    nc.scalar.activation(st[:, g, :], pp[:], AF.Identity,
                                         bias=bout_sb[:, v:v + 1])
                else:
                    nc.vector.tensor_scalar_add(st[:, g, :], pp[:],
                                                bout_sb[:, v:v + 1])
                if g == 3 or v == NV - 1:
                    v0 = (v // 4) * 4
                    nc.sync.dma_start(outT_d[:, v0:v + 1, :], st[:])

    nc.compile()
    return nc


def prep_inputs(features, captions, embed_table, W_ih, W_hh, b_ih, b_hh,
                W_out, b_out):
    """Host-side shard + layout prep. Returns per-core input maps."""
    bf = ml_dtypes.bfloat16
    features = np.asarray(features, dtype=np.float32)
    captions = np.asarray(captions).astype(np.int64)
    embed_table = np.asarray(embed_table, dtype=np.float32)
    W_ih = np.asarray(W_ih, dtype=np.float32)
    W_hh = np.asarray(W_hh, dtype=np.float32)
    b_ih = np.asarray(b_ih, dtype=np.float32)
    b_hh = np.asarray(b_hh, dtype=np.float32)
    W_out = np.asarray(W_out, dtype=np.float32)
    b_out = np.asarray(b_out, dtype=np.float32)

    perm = _gate_perm()

    emb = np.zeros((V, EPAD), dtype=bf)
    emb[:, :E] = embed_table.astype(bf)
    emb[:, EPAD - 1] = bf(1.0)

    wih = np.zeros((EPAD, 4 * H), dtype=np.float32)
    wih[:E, :] = W_ih.T[:, perm]
    wih[EPAD - 1, :] = (b_ih + b_hh)[perm]
    wih = wih.astype(bf).reshape(3, 128, 4, 512)

    whh = np.ascontiguousarray(W_hh.T[:, perm]).astype(bf).reshape(4, 128, 4, 512)

    wout = np.zeros((H, VPAD), dtype=np.float32)
    wout[:, :V] = W_out.T
    wout = wout.astype(bf).reshape(4, 128, NV, 128)

    boutp = np.zeros((VPAD,), dtype=np.float32)
    boutp[:V] = b_out
    bout_r = np.ascontiguousarray(boutp.reshape(NV, 128).T)

    idf = np.eye(128, dtype=np.float32)
    i16b = np.eye(16, dtype=bf)

    shared = dict(emb=emb, wih=wih, whh=whh, wout=wout, bout=bout_r,
                  idf=idf, i16b=i16b)

    in_maps = []
    for c in range(NCORES):
        cap_c = captions[c * BC:(c + 1) * BC]                 # [16, 20]
        # idx block [16, NIDX//16], replicated into all 8 GpSimd core groups
        blk = np.zeros((16, NIDX // 16), dtype=np.int16)
        blk[:, :T] = cap_c.astype(np.int16)
        idx = np.tile(blk, (8, 1))
        feat_c = features[c * BC:(c + 1) * BC]                # [16, 512]
        h0t = np.ascontiguousarray(
            feat_c.reshape(BC, 4, 128).transpose(2, 1, 0)).astype(bf)
        in_maps.append(dict(shared, idx=idx, h0t=h0t))
    return in_maps


def unshard(core_outs):
    """core_outs: list of 8 arrays [NV, 128, NT] f32 -> full [B, T, V]."""
    parts = []
    for o in core_outs:
        o = np.asarray(o, dtype=np.float32)          # [128, NV, NT]
        o = o.transpose(1, 0, 2).reshape(VPAD, NT)[:V]             # [V, 320]
        parts.append(o.reshape(V, T, BC).transpose(2, 1, 0))       # [16, T, V]
    return np.ascontiguousarray(np.concatenate(parts, axis=0))


_NC_CACHE = {}


def kernel(**inputs) -> np.ndarray:
    from concourse.bass_utils import run_bass_kernel_spmd

    if "nc" not in _NC_CACHE:
        _NC_CACHE["nc"] = build_nc()
    nc = _NC_CACHE["nc"]

    in_maps = prep_inputs(**inputs)
    res = run_bass_kernel_spmd(nc, in_maps, core_ids=list(range(NCORES)))
    return unshard([res.results[c]["outT"] for c in range(NCORES)])


# revision 18
# speedup vs baseline: 1.1297x; 1.0683x over previous
"""DecoderLSTM Trainium2 kernel.

Computes, for inputs matching the reference nn module:
    x  = embed_table[captions]                      # [B, T, E]
    xg = einsum('bte,ge->tbg', x, W_ih) + b_ih + b_hh
    (h, c) LSTM scan over T steps, h0 = features, c0 = 0
    out = einsum('tbh,vh->btv', hs, W_out) + b_out  # [B, T, V]

Sharding: data-parallel over batch. 8 cores x 16 batch rows each.
Weights are replicated (cast to bf16 host-side); each core computes its
16-row slice of the output. Per-core output is produced in transposed
layout [V, T*Bc] and untransposed on the host during unshard.

Device layout notes (per core, Bc = 16 batch rows):
  - Embedding gather: dma_gather(transpose=True) pulls the 320 caption
    rows of the bf16 [V, 384]-padded table directly into x_T layout
    [128p=E-offset, 3=E-block, 384=(t,b) col]. Table column 383 is 1.0
    so row 383 of W_ihT carries (b_ih + b_hh): bias folded into the
    xg matmul.
  - Gate permutation: the 4H=2048 gate dim is reordered host-side so
    column-group j of the recurrent matmul computes
    [i_j | f_j | g_j | o_j] (H-slice j of each gate). Gates land in one
    PSUM bank as [128=(32j+b), 4, 128] and the whole nonlinearity runs
    on [128, *] tiles.
  - Recurrent matmul: 4 concurrent column-tiled matmuls (tile_position
    (0, 32j), M=16) stream W_hhT chunks; each group's accumulation is
    seeded by an identity-matmul that injects xg_t (start=True).
  - h_T for the next step comes from 4 row-tiled PE transposes of the
    [16@32k, 128] slices of h.
  - Projection: out_T = W_out @ hs_T with W_out blocks stationary,
    V on partitions; b_out added during PSUM evacuation via the ACT
    per-partition bias. Chunked in two so the first half overlaps the
    recurrence.
"""

import numpy as np
import ml_dtypes

import concourse.bass as bass
import concourse.mybir as mybir
import concourse.tile as tile
from concourse import bacc

BF16 = mybir.dt.bfloat16
F32 = mybir.dt.float32
I16 = mybir.dt.int16

B, T, E, H, V = 128, 20, 300, 512, 10000
EPAD = 384            # E padded; col 383 is the ones column (bias row)
NCORES = 8
BC = B // NCORES      # 16 batch rows per core
NT = BC * T           # 320 (t,b) columns per core
NIDX = 384            # gather idx count (padded to %128)
NV = 79               # ceil(10112 / 128) vocab row-tiles
VPAD = NV * 128       # 10112
AF = mybir.ActivationFunctionType


def _gate_perm():
    """new gate-dim order: chunk j = [i_j | f_j | g_j | o_j], blocks of 128."""
    perm = np.empty(4 * H, dtype=np.int64)
    n = 0
    for j in range(4):
        for q in range(4):          # i, f, g, o (PyTorch LSTM order)
            for r in range(128):
                perm[n] = q * H + j * 128 + r
                n += 1
    return perm


def build_nc():
    nc = bacc.Bacc("TRN2", target_bir_lowering=False, debug=False)

    # ---- DRAM parameters (per-core shapes) ----
    emb_d = nc.dram_tensor("emb", [V, EPAD], BF16, kind="ExternalInput")
    idx_d = nc.dram_tensor("idx", [128, NIDX // 16], I16, kind="ExternalInput")
    wih_d = nc.dram_tensor("wih", [3, 128, 4, 512], BF16, kind="ExternalInput")
    whh_d = nc.dram_tensor("whh", [4, 128, 4, 512], BF16, kind="ExternalInput")
    wout_d = nc.dram_tensor("wout", [4, 128, NV, 128], BF16, kind="ExternalInput")
    bout_d = nc.dram_tensor("bout", [128, NV], F32, kind="ExternalInput")
    h0t_d = nc.dram_tensor("h0t", [128, 4, BC], BF16, kind="ExternalInput")
    idf_d = nc.dram_tensor("idf", [128, 128], F32, kind="ExternalInput")
    i16b_d = nc.dram_tensor("i16b", [16, 16], BF16, kind="ExternalInput")
    outT_d = nc.dram_tensor("outT", [128, NV, NT], F32, kind="ExternalOutput")

    with tile.TileContext(nc) as tc:
        with (
            tc.tile_pool(name="const", bufs=1) as const,
            tc.tile_pool(name="wpool", bufs=1) as wpool,
            tc.tile_pool(name="xgstep", bufs=3) as xgstep_p,
            tc.tile_pool(name="work", bufs=2) as work,
            tc.tile_pool(name="stage", bufs=4) as stage_p,
            tc.tile_pool(name="psg", bufs=1, space="PSUM") as ps_gates,
            tc.tile_pool(name="psh", bufs=1, space="PSUM") as ps_ht,
            tc.tile_pool(name="psb", bufs=2, space="PSUM") as ps_big,
        ):
            idx_sb = const.tile([128, NIDX // 16], I16, tag="idx")
            i16b_sb = const.tile([16, 16], BF16, tag="i16b")
            idf_sb = const.tile([128, 128], F32, tag="idf")
            bout_sb = const.tile([128, NV], F32, tag="bout")

            xT = wpool.tile([128, 3, EPAD], BF16, tag="xT")
            wih_sb = [wpool.tile([128, 4, 512], BF16, tag=f"wih{k}", name=f"wih{k}") for k in range(3)]
            whh_sb = [wpool.tile([128, 4, 512], BF16, tag=f"whh{k}", name=f"whh{k}") for k in range(4)]
            wout_sb = [wpool.tile([128, NV, 128], BF16, tag=f"wo{k}", name=f"wo{k}") for k in range(4)]
            xg_sb = [wpool.tile([128, 4, 512], BF16, tag=f"xg{m}", name=f"xg{m}") for m in range(3)]
            # hs_T: slot s holds h after step s-1 (slot 0 = h0), packed
            # [128p=H-offset, slot, H-block k, b]
            hsT = wpool.tile([128, T + 1, 4, BC], BF16, tag="hsT")
            C = wpool.tile([128, 128], F32, tag="C")

            gates_ps = ps_gates.tile([128, 4, 128], F32, tag="g")
            ht_ps = ps_ht.tile([128, 4, 32], F32, tag="ht")

            # ---- loads ----
            # sync HWDGE ring: only small latency-critical transfers (idx,
            # h0t, per-step xg rebases). scalar HWDGE ring: bulk weights.
            # HWDGE executes FIFO per issuing engine, so big weight loads
            # must not sit in front of the per-step rebase DMAs.
            nc.sync.dma_start(idx_sb[:], idx_d[:])
            nc.sync.dma_start(hsT[:, 0, :, :], h0t_d[:])
            nc.gpsimd.dma_gather(xT[:], emb_d[:], idx_sb[:], NIDX, NIDX, EPAD,
                                 transpose=True)
            nc.scalar.dma_start(i16b_sb[:], i16b_d[:])
            nc.scalar.dma_start(idf_sb[:], idf_d[:])
            nc.scalar.dma_start(bout_sb[:], bout_d[:])
            for k in range(3):
                nc.scalar.dma_start(wih_sb[k][:], wih_d[k])
            for k in range(4):
                nc.scalar.dma_start(whh_sb[k][:], whh_d[k])
            nc.vector.memset(gates_ps[:], 0.0)
            nc.vector.memset(C[:], 0.0)
            for k in range(4):
                nc.scalar.dma_start(wout_sb[k][:], wout_d[k])

            # ---- xg = x @ W_ihT -> [(t,b) rows, 2048 perm'd gate cols] ----
            # m-tile 0 runs up front (needed at step 0); m1/m2 n-groups are
            # emitted inside steps 0..7 as PE filler during the act windows.
            def emit_xg(m, n):
                ps = ps_big.tile([128, 512], F32, tag="big")
                for k in range(3):
                    nc.tensor.matmul(
                        ps[:],
                        xT[:, k, m * 128:(m + 1) * 128],
                        wih_sb[k][:, n, :],
                        start=(k == 0), stop=(k == 2),
                    )
                if n % 2 == 0:
                    nc.vector.tensor_copy(xg_sb[m][:, n, :], ps[:])
                else:
                    nc.scalar.copy(xg_sb[m][:, n, :], ps[:])

            for n in range(4):
                emit_xg(0, n)

            def emit_xgt_fetch(t):
                # rebase step-t xg rows to partition base 0 (prefetched)
                m, r0 = t // 8, (t % 8) * BC
                xg_t = xgstep_p.tile([BC, 4, 512], BF16, tag="xgt",
                                     name=f"xgt{t}")
                nc.sync.dma_start(xg_t[:], xg_sb[m][r0:r0 + BC, :, :])
                return xg_t

            # ---- recurrence ----
            xg_fetched = [emit_xgt_fetch(0), emit_xgt_fetch(1)]
            for t in range(T):
                xg_t = xg_fetched[t]
                if t + 2 < T:
                    xg_fetched.append(emit_xgt_fetch(t + 2))

                # gate matmuls: 4 concurrent column groups
                for j in range(4):
                    nc.tensor.matmul(
                        gates_ps[32 * j:32 * j + BC, :, :],
                        i16b_sb[:],
                        xg_t[:, j, :],
                        start=True, stop=False,
                        tile_position=(0, 32 * j),
                        skip_group_check=True,
                    )
                for k in range(4):
                    for j in range(4):
                        nc.tensor.matmul(
                            gates_ps[32 * j:32 * j + BC, :, :],
                            hsT[:, t, k, :],
                            whh_sb[k][:, j, :],
                            start=False, stop=(k == 3),
                            tile_position=(0, 32 * j),
                            skip_group_check=True,
                        )

                # nonlinearity: A = [sig(i), sig(f), tanh(g), sig(o)]
                A = work.tile([128, 4, 128], F32, tag="A")
                nc.scalar.activation(A[:, 0:2, :], gates_ps[:, 0:2, :], AF.Sigmoid)
                nc.scalar.activation(A[:, 2, :], gates_ps[:, 2, :], AF.Tanh)
                nc.scalar.activation(A[:, 3, :], gates_ps[:, 3, :], AF.Sigmoid)
                T2 = work.tile([128, 128], F32, tag="T2")
                T1 = work.tile([128, 128], F32, tag="T1")
                TC = work.tile([128, 128], F32, tag="TC")
                Hn = work.tile([128, 128], F32, tag="Hn")
                nc.vector.tensor_mul(T2[:], A[:, 0, :], A[:, 2, :])   # i*g
                nc.vector.tensor_mul(T1[:], A[:, 1, :], C[:])         # f*c
                nc.vector.tensor_add(C[:], T1[:], T2[:])
                nc.scalar.activation(TC[:], C[:], AF.Tanh)
                nc.vector.tensor_mul(Hn[:], A[:, 3, :], TC[:])        # o*tanh(c)

                # PE filler during the act window: xg m1/m2 groups (each
                # ~1.3us of PE work, inside the ~2.5us nonlinearity window)
                if t < 4:
                    emit_xg(1, t)
                elif t < 8:
                    emit_xg(2, t - 4)

                # h -> h_T: one full 128x128 PE transpose; the four h_T
                # blocks are ht_ps[:, k, 0:16] (garbage lands in cols 16:32)
                nc.tensor.transpose(ht_ps[:], Hn[:], idf_sb[:])
                nc.vector.tensor_copy(hsT[:, t + 1, :, :], ht_ps[:, :, 0:BC])

            # ---- projection tail: out_T = W_out @ hs_T, all 20 slots ----
            # (N=320 streams keep LDWEIGHTS fully hidden; PE is warm here)
            st = None
            for v in range(NV):
                pp = ps_big.tile([128, NT], F32, tag="big")
                for k in range(4):
                    nc.tensor.matmul(
                        pp[:],
                        wout_sb[k][:, v, :],
                        hsT[:, 1:T + 1, k, :],
                        start=(k == 0), stop=(k == 3),
                    )
                g = v % 4
                if g == 0:
                    nv = min(4, NV - v)
                    st = stage_p.tile([128, nv, NT], F32, tag="st",
                                      name=f"st{v}")
                if v % 2 == 0:
                    nc.scalar.activation(st[:, g, :], pp[:], AF.Identity,
                                         bias=bout_sb[:, v:v + 1])
                else:
                    nc.vector.tensor_scalar_add(st[:, g, :], pp[:],
                                                bout_sb[:, v:v + 1])
                if g == 3 or v == NV - 1:
                    v0 = (v // 4) * 4
                    nc.sync.dma_start(outT_d[:, v0:v + 1, :], st[:])

    nc.compile()
    return nc


def prep_inputs(features, captions, embed_table, W_ih, W_hh, b_ih, b_hh,
                W_out, b_out):
    """Host-side shard + layout prep. Returns per-core input maps."""
    bf = ml_dtypes.bfloat16
    features = np.asarray(features, dtype=np.float32)
    captions = np.asarray(captions).astype(np.int64)
    embed_table = np.asarray(embed_table, dtype=np.float32)
    W_ih = np.asarray(W_ih, dtype=np.float32)
    W_hh = np.asarray(W_hh, dtype=np.float32)
    b_ih = np.asarray(b_ih, dtype=np.float32)
    b_hh = np.asarray(b_hh, dtype=np.float32)
    W_out = np.asarray(W_out, dtype=np.float32)
    b_out = np.asarray(b_out, dtype=np.float32)

    perm = _gate_perm()

    emb = np.zeros((V, EPAD), dtype=bf)
    emb[:, :E] = embed_table.astype(bf)
    emb[:, EPAD - 1] = bf(1.0)

    wih = np.zeros((EPAD, 4 * H), dtype=np.float32)
    wih[:E, :] = W_ih.T[:, perm]
    wih[EPAD - 1, :] = (b_ih + b_hh)[perm]
    wih = wih.astype(bf).reshape(3, 128, 4, 512)

    whh = np.ascontiguousarray(W_hh.T[:, perm]).astype(bf).reshape(4, 128, 4, 512)

    wout = np.zeros((H, VPAD), dtype=np.float32)
    wout[:, :V] = W_out.T
    wout = wout.astype(bf).reshape(4, 128, NV, 128)

    boutp = np.zeros((VPAD,), dtype=np.float32)
    boutp[:V] = b_out
    bout_r = np.ascontiguousarray(boutp.reshape(NV, 128).T)

    idf = np.eye(128, dtype=np.float32)
    i16b = np.eye(16, dtype=bf)

    shared = dict(emb=emb, wih=wih, whh=whh, wout=wout, bout=bout_r,
                  idf=idf, i16b=i16b)

    in_maps = []
    for c in range(NCORES):
        cap_c = captions[c * BC:(c + 1) * BC]                 # [16, 20]
        # idx block [16, NIDX//16], replicated into all 8 GpSimd core groups
        blk = np.zeros((16, NIDX // 16), dtype=np.int16)
        blk[:, :T] = cap_c.astype(np.int16)
        idx = np.tile(blk, (8, 1))
        feat_c = features[c * BC:(c + 1) * BC]                # [16, 512]
        h0t = np.ascontiguousarray(
            feat_c.reshape(BC, 4, 128).transpose(2, 1, 0)).astype(bf)
        in_maps.append(dict(shared, idx=idx, h0t=h0t))
    return in_maps


def unshard(core_outs):
    """core_outs: list of 8 arrays [NV, 128, NT] f32 -> full [B, T, V]."""
    parts = []
    for o in core_outs:
        o = np.asarray(o, dtype=np.float32)          # [128, NV, NT]
        o = o.transpose(1, 0, 2).reshape(VPAD, NT)[:V]             # [V, 320]
        parts.append(o.reshape(V, T, BC).transpose(2, 1, 0))       # [16, T, V]
    return np.ascontiguousarray(np.concatenate(parts, axis=0))


_NC_CACHE = {}


def kernel(**inputs) -> np.ndarray:
    from concourse.bass_utils import run_bass_kernel_spmd

    if "nc" not in _NC_CACHE:
        _NC_CACHE["nc"] = build_nc()
    nc = _NC_CACHE["nc"]

    in_maps = prep_inputs(**inputs)
    res = run_bass_kernel_spmd(nc, in_maps, core_ids=list(range(NCORES)))
    return unshard([res.results[c]["outT"] for c in range(NCORES)])
